# revision 1
# baseline (speedup 1.0000x reference)
"""Trainium2 Bass kernel for nn_NeighborhoodAttentionModule.

Pure data-parallel over batch: B=16384 rows split as 2048 rows/core across 8
NeuronCores. Per core, per 128-row b-tile:

  scores   zT[(h,a), (b,k)] = VU.T @ neT          (PE, fp16, fp32 PSUM)
  s1T[(h,a), b] = U2.T @ ceT                      (PE)
  h = tanh(z + s1 + b1)                           (ACT copy, GPSIMD add, ACT tanh)
  raw[(b,k), h] = h_chunk.T @ w2pair              (PE, chunk-stationary)
  em = exp(raw + nw) * valid                      (ACT exp + DVE mults, fp16)
  expblk[(b,k), (c,b',h)] = em * blockmask        (DVE, one op per b-tile)
  agg_u[(b',h), d] (+ S col) = expblk_c.T @ ne_row_c   (PE, per chunk)
  agg_n = agg_u * recip(S + eps)                  (ACT per-partition scale)
  aggT = PE-transpose(agg_n)                      (PE + DVE copies)
  fused[b, d'] = sum_(h,dh) aggT_slice.T @ Wcc    (PE, + bc row)
  out = LayerNorm(center + fused)                 (DVE/ACT)

All host-side work is layout/dtype transformation only (fp16 casts, transposes,
weight folding: VU = W1b-W1c, U2 = W1a+W1c, Wc*sigmoid(alpha)).
"""
import os
import numpy as np

B, K, D, H, A = 16384, 16, 256, 2, 64
NCORES = 8
BC = B // NCORES      # rows per core
NBT = BC // 128       # b-tiles per core (16)
NCH = 16              # chunks (128 rows) per b-tile
HA = H * A            # 128
EPS = 1e-5

LAST_EXEC_NS = None

_prog_cache = {}


def _maybe_install_profile_hook():
    """Optional NTFF profiling hook (for local testing only; fails soft)."""
    import sys, types, contextlib, ctypes

    if "antenv.axon_hooks" in sys.modules:
        return
    try:
        mod = types.ModuleType("antenv.axon_hooks")
        _state = {"hook": None}
        mod.set_axon_ntff_profile_hook = lambda h: _state.__setitem__("hook", h)
        mod.get_axon_ntff_profile_hook = lambda: _state["hook"]
        sys.modules["antenv.axon_hooks"] = mod
        import antenv

        antenv.axon_hooks = mod
        so_path = "/opt/axon/libaxon_pjrt.so"
        lib = ctypes.CDLL(so_path)
        if not hasattr(lib, "axon_start_nrt_profile"):
            return
        lib.axon_start_nrt_profile.argtypes = [
            ctypes.POINTER(ctypes.c_int64),
            ctypes.c_size_t,
        ]
        lib.axon_start_nrt_profile.restype = ctypes.c_int64
        lib.axon_stop_nrt_profile.argtypes = [ctypes.c_char_p]
        lib.axon_stop_nrt_profile.restype = ctypes.c_int64

        @contextlib.contextmanager
        def _hook(output_dir, device_ids):
            import jax

            jax.devices()
            if device_ids:
                ids = (ctypes.c_int64 * len(device_ids))(*device_ids)
                rc = lib.axon_start_nrt_profile(ids, len(device_ids))
            else:
                rc = lib.axon_start_nrt_profile(None, 0)
            if rc != 0:
                raise RuntimeError(f"axon_start_nrt_profile rc={rc}")
            try:
                yield
            finally:
                n = lib.axon_stop_nrt_profile(str(output_dir).encode())
                print(f"profile: {n} ntff file(s) -> {output_dir}")

        mod.set_axon_ntff_profile_hook(_hook)
    except Exception as e:  # noqa: BLE001
        print("profile hook unavailable:", e)


def _build_program(apply_gamma_beta: bool, apply_b1: bool):
    from concourse import bacc, tile, mybir

    F16 = mybir.dt.float16
    F32 = mybir.dt.float32
    AFT = mybir.ActivationFunctionType
    AX = mybir.AxisListType

    nc = bacc.Bacc(None, target_bir_lowering=False)

    # ---- DRAM parameters (per-core shard) ----
    dp = nc.declare_dram_parameter
    neT_d = dp("neT", [2, 128, BC * K], F16, isOutput=False)       # [dh, dd, col]
    ner_d = dp("ner", [BC * K, D], F16, isOutput=False)            # row-major
    ceT_d = dp("ceT", [2, 128, BC], F16, isOutput=False)           # [dh, dd, b]
    cen_d = dp("center", [BC, D], F32, isOutput=False)
    nw_d = dp("nw_t", [NBT, 128, NCH], F32, isOutput=False)        # [t][p, c]
    va_d = dp("valid_t", [NBT, 128, NCH], F16, isOutput=False)
    vu_d = dp("VU", [2, 128, HA], F16, isOutput=False)
    u2_d = dp("U2", [2, 128, HA], F16, isOutput=False)
    w2_d = dp("w2pair", [128, H], F16, isOutput=False)
    b1_d = dp("b1cat", [128, 1], F32, isOutput=False)
    bm_d = dp("bm16", [128, 8, H], F16, isOutput=False)
    wcc_d = dp("Wcc", [2, 2, 128, D], F16, isOutput=False)         # [h, dh, dd, d']
    bcr_d = dp("bc_row", [1, D], F16, isOutput=False)
    one_r_d = dp("ones_row", [1, 128], F16, isOutput=False)
    one_c_d = dp("ones_col", [128, 1], F16, isOutput=False)
    id_d = dp("ident", [128, 128], F16, isOutput=False)
    gam_d = dp("gamma_r", [1, D], F32, isOutput=False)
    bet_d = dp("beta_r", [1, D], F32, isOutput=False)
    out_d = dp("out", [BC, D], F32, isOutput=True)

    with tile.TileContext(nc) as tc:
        with (
            tc.tile_pool(name="const", bufs=1) as cpool,
            tc.tile_pool(name="loads", bufs=3) as lpool,
            tc.tile_pool(name="work", bufs=2) as wpool,
            tc.tile_pool(name="zps", bufs=2, space="PSUM") as zps_p,
            tc.tile_pool(name="s1ps", bufs=1, space="PSUM") as s1ps_p,
            tc.tile_pool(name="rawps", bufs=1, space="PSUM") as rawps_p,
            tc.tile_pool(name="aggps", bufs=1, space="PSUM") as aggps_p,
            tc.tile_pool(name="trps", bufs=1, space="PSUM") as trps_p,
            tc.tile_pool(name="fups", bufs=1, space="PSUM") as fups_p,
        ):
            # ---- constants to SBUF (once) ----
            def cload(name, dram_ap, shape, dt):
                t = cpool.tile(shape, dt, tag=name, name=name)
                nc.sync.dma_start(t[:], dram_ap)
                return t

            vu = [cload(f"vu{i}", vu_d[i], [128, HA], F16) for i in range(2)]
            u2 = [cload(f"u2{i}", u2_d[i], [128, HA], F16) for i in range(2)]
            ceT = [cload(f"ceT{i}", ceT_d[i], [128, BC], F16) for i in range(2)]
            w2p = cload("w2p", w2_d[:], [128, H], F16)
            b1c = cload("b1c", b1_d[:], [128, 1], F32)
            bm16 = cload("bm16", bm_d[:], [128, 8, H], F16)
            wcc = [
                [cload(f"wcc{h}{dh}", wcc_d[h, dh], [128, D], F16) for dh in range(2)]
                for h in range(2)
            ]
            bcr = cload("bcr", bcr_d[:], [1, D], F16)
            ones_r = cload("ones_r", one_r_d[:], [1, 128], F16)
            ones_c = cload("ones_c", one_c_d[:], [128, 1], F16)
            ident = cload("ident", id_d[:], [128, 128], F16)
            gam_t = (
                cload("gam", gam_d[:].to_broadcast((128, D)), [128, D], F32)
                if apply_gamma_beta else None
            )
            bet_t = (
                cload("bet", bet_d[:].to_broadcast((128, D)), [128, D], F32)
                if apply_gamma_beta else None
            )
            eps_t = cpool.tile([128, 1], F32, tag="eps")
            nc.vector.memset(eps_t[:], EPS)

            for t in range(NBT):
                col0 = t * 2048  # neT/z column base for this b-tile (2048 cols)

                # ---- loads ----
                neT = [lpool.tile([128, 2048], F16, tag=f"neT{i}", name=f"neT{i}") for i in range(2)]
                for hh in range(2):
                    nc.sync.dma_start(neT[hh][:], neT_d[hh, :, col0:col0 + 2048])
                ner = lpool.tile([128, NCH, D], F16, tag="ner")
                nc.sync.dma_start(
                    ner[:],
                    ner_d[col0:col0 + 2048, :].rearrange("(c p) d -> p c d", p=128),
                )
                cen_t = lpool.tile([128, D], F32, tag="cen")
                nc.sync.dma_start(cen_t[:], cen_d[t * 128:(t + 1) * 128, :])
                nw_t = lpool.tile([128, NCH], F32, tag="nw")
                nc.sync.dma_start(nw_t[:], nw_d[t])
                va_t = lpool.tile([128, NCH], F16, tag="va")
                nc.sync.dma_start(va_t[:], va_d[t])

                # ---- s1T = U2.T @ ceT (per b-tile slice) ----
                s1_ps = s1ps_p.tile([128, 128], F32)
                for hh in range(2):
                    nc.tensor.matmul(
                        s1_ps[:], u2[hh][:], ceT[hh][:, t * 128:(t + 1) * 128],
                        start=(hh == 0), stop=(hh == 1),
                    )
                s1_sb = wpool.tile([128, 128], F16, tag="s1sb")
                if apply_b1:
                    nc.scalar.activation(s1_sb[:], s1_ps[:], AFT.Identity, bias=b1c[:])
                else:
                    nc.scalar.copy(s1_sb[:], s1_ps[:])

                # ---- scores zT + s1 + tanh -> h ----
                h_t = wpool.tile([128, 2048], F16, tag="h")
                for c4 in range(4):
                    z_ps = zps_p.tile([128, 512], F32, tag="z")
                    for hh in range(2):
                        nc.tensor.matmul(
                            z_ps[:], vu[hh][:], neT[hh][:, c4 * 512:(c4 + 1) * 512],
                            start=(hh == 0), stop=(hh == 1),
                        )
                    hs = h_t[:, c4 * 512:(c4 + 1) * 512]
                    # (z + s1-broadcast) -> fp16 SBUF in one DVE op
                    s1b = s1_sb[:, c4 * 32:(c4 + 1) * 32][:, :, None].to_broadcast((128, 32, 16))
                    nc.vector.tensor_add(
                        hs.rearrange("p (b k) -> p b k", k=16),
                        z_ps[:].rearrange("p (b k) -> p b k", k=16),
                        s1b,
                    )
                    # tanh in place on ACT
                    nc.scalar.activation(hs, hs, AFT.Tanh)

                # ---- raw scores (chunk-stationary matmuls) ----
                raw_ps = rawps_p.tile([128, 36], F32)
                for c in range(NCH):
                    nc.tensor.matmul(
                        raw_ps[:, 2 * c:2 * c + 2],
                        h_t[:, c * 128:(c + 1) * 128], w2p[:],
                        start=True, stop=True,
                    )

                # ---- em = exp(raw) * exp(nw) * valid ----
                expnw = wpool.tile([128, NCH], F16, tag="expnw")
                nc.scalar.activation(expnw[:], nw_t[:], AFT.Exp)
                expnwv = wpool.tile([128, NCH], F16, tag="expnwv")
                nc.vector.tensor_mul(expnwv[:], expnw[:], va_t[:])
                exp_t = wpool.tile([128, NCH, H], F16, tag="expt")
                nc.scalar.activation(
                    exp_t[:].rearrange("p c h -> p (c h)"),
                    raw_ps[:, 0:32], AFT.Exp,
                )
                em = wpool.tile([128, NCH, H], F16, tag="em")
                nc.vector.tensor_mul(
                    em[:], exp_t[:], expnwv[:, :, None].to_broadcast((128, NCH, H))
                )
                # expblk[(b,k), (c, b', h)] = em * blockmask
                expblk = wpool.tile([128, NCH, 8, H], F16, tag="expblk")
                nc.vector.tensor_mul(
                    expblk[:],
                    em[:, :, None, :].to_broadcast((128, NCH, 8, H)),
                    bm16[:, None, :, :].to_broadcast((128, NCH, 8, H)),
                )

                # ---- aggregation (+ S in raw_ps cols 32..35) ----
                agg_ps = [
                    aggps_p.tile([128, 2, D], F32, tag=f"agg{i}", name=f"agg{i}")
                    for i in range(2)
                ]
                for c in range(NCH):
                    q, j = c // 4, c % 4
                    lhs = expblk[:, c]
                    nc.tensor.matmul(
                        agg_ps[q // 2][32 * j:32 * j + 16, q % 2, :],
                        lhs, ner[:, c], start=True, stop=True,
                        tile_position=(0, 32 * j),
                    )
                    nc.tensor.matmul(
                        raw_ps[32 * j:32 * j + 16, 32 + q:33 + q], lhs, ones_c[:],
                        start=True, stop=True, tile_position=(0, 32 * j),
                    )

                # recip(S + tiny)
                s_eps = wpool.tile([128, 4], F32, tag="seps")
                nc.vector.tensor_scalar_add(s_eps[:], raw_ps[:, 32:36], 1e-30)
                recs = wpool.tile([128, 4], F32, tag="recs")
                nc.vector.reciprocal(recs[:], s_eps[:])

                # agg_n = agg_u * recipS (ACT per-partition scale, fp16 out)
                agg_n = wpool.tile([128, 4, D], F16, tag="aggn")
                for q in range(4):
                    nc.scalar.mul(
                        agg_n[:, q, :], agg_ps[q // 2][:, q % 2, :], recs[:, q:q + 1]
                    )

                # ---- transpose agg_n -> aggT[dh] [dd, (q, 32j+m)] ----
                aggT = []
                for dh in range(2):
                    tr_ps = trps_p.tile([128, 4, 128], F16, tag="trps")
                    for q in range(4):
                        nc.tensor.transpose(
                            tr_ps[:, q, :],
                            agg_n[:, q, dh * 128:(dh + 1) * 128],
                            ident[:],
                        )
                    # reorder (q, 32j + 2b' + h) -> linear (h, q, j, b') during the
                    # PSUM->SBUF copy so matmul weight APs stay 2D
                    a_t = wpool.tile([128, 2, 128], F16, tag=f"aggT{dh}", name=f"aggT{dh}")
                    src_v = (
                        tr_ps[:]
                        .rearrange(
                            "p q (j half b two) -> p q j half b two",
                            j=4, half=2, b=8, two=2,
                        )[:, :, :, 0, :, :]
                        .rearrange("p q j b two -> p two q j b")
                    )
                    dst_v = a_t[:].rearrange("p two (q j b) -> p two q j b", q=4, j=4, b=8)
                    nc.vector.tensor_copy(dst_v, src_v)
                    aggT.append(a_t)

                # ---- fused = combined @ Wc (+ bc row) ----
                fu_ps = fups_p.tile([128, D], F32)
                mms = [(h, dh) for h in range(2) for dh in range(2)]
                for i, (h, dh) in enumerate(mms):
                    lhs = aggT[dh][:, h, :]
                    nc.tensor.matmul(
                        fu_ps[:], lhs, wcc[h][dh][:],
                        start=(i == 0), stop=False,
                    )
                nc.tensor.matmul(fu_ps[:], ones_r[:], bcr[:], start=False, stop=True)

                # ---- residual + layernorm ----
                x_t = wpool.tile([128, D], F32, tag="x")
                nc.vector.tensor_add(x_t[:], fu_ps[:], cen_t[:])
                msum = wpool.tile([128, 1], F32, tag="msum")
                nc.vector.reduce_sum(msum[:], x_t[:], axis=AX.X)
                negmean = wpool.tile([128, 1], F32, tag="negmean")
                nc.scalar.mul(negmean[:], msum[:], -1.0 / D)
                xc = wpool.tile([128, D], F32, tag="xc")
                nc.gpsimd.tensor_scalar_add(xc[:], x_t[:], negmean[:])
                sq = wpool.tile([128, D], F32, tag="sq")
                nc.gpsimd.tensor_mul(sq[:], xc[:], xc[:])
                vsum = wpool.tile([128, 1], F32, tag="vsum")
                nc.vector.reduce_sum(vsum[:], sq[:], axis=AX.X)
                stdev = wpool.tile([128, 1], F32, tag="stdev")
                nc.scalar.activation(stdev[:], vsum[:], AFT.Sqrt, bias=eps_t[:], scale=1.0 / D)
                invstd = wpool.tile([128, 1], F32, tag="invstd")
                nc.vector.reciprocal(invstd[:], stdev[:])
                xn = wpool.tile([128, D], F32, tag="xn")
                nc.gpsimd.tensor_scalar_mul(xn[:], xc[:], invstd[:])
                if apply_gamma_beta:
                    nc.vector.tensor_mul(xn[:], xn[:], gam_t[:])
                    nc.vector.tensor_add(xn[:], xn[:], bet_t[:])
                nc.sync.dma_start(out_d[t * 128:(t + 1) * 128, :], xn[:])

    nc.finalize()
    return nc


def kernel(center_emb, neighbor_embs, neighbor_weights, neighbor_valid,
           W1, b1, w2, Wc, bc, alpha, gamma, beta):
    from concourse.bass_utils import run_bass_kernel_spmd

    global LAST_EXEC_NS

    f32 = np.float32
    f16 = np.float16
    ce = np.asarray(center_emb, f32)
    ne = np.asarray(neighbor_embs, f32)
    nw = np.asarray(neighbor_weights, f32)
    va = np.asarray(neighbor_valid)
    W1 = np.asarray(W1, f32)
    b1 = np.asarray(b1, f32)
    w2 = np.asarray(w2, f32)
    Wc = np.asarray(Wc, f32)
    bc = np.asarray(bc, f32)
    alpha = np.asarray(alpha, f32)
    gamma = np.asarray(gamma, f32)
    beta = np.asarray(beta, f32)

    apply_gamma_beta = not (np.all(gamma == 1.0) and np.all(beta == 0.0))
    apply_b1 = bool(np.any(b1 != 0.0))

    key = (apply_gamma_beta, apply_b1)
    if key not in _prog_cache:
        _prog_cache[key] = _build_program(*key)
    nc = _prog_cache[key]

    # ---- host-side const prep (weight folding + fp16 casts) ----
    sig = 1.0 / (1.0 + np.exp(-float(alpha[0])))
    VU = np.concatenate([W1[h, D:2 * D] - W1[h, 2 * D:3 * D] for h in range(H)], axis=1)
    U2 = np.concatenate([W1[h, :D] + W1[h, 2 * D:3 * D] for h in range(H)], axis=1)
    vu_in = np.ascontiguousarray(VU.reshape(2, 128, HA).astype(f16))
    u2_in = np.ascontiguousarray(U2.reshape(2, 128, HA).astype(f16))
    w2pair = np.zeros((128, H), f16)
    for h in range(H):
        w2pair[h * A:(h + 1) * A, h] = w2[h].astype(f16)
    b1cat = b1.reshape(HA, 1).astype(f32)
    bm16 = np.zeros((128, 8, H), f16)
    for p in range(128):
        bm16[p, p // 16, :] = 1.0
    wcc = np.ascontiguousarray(
        (Wc * sig).astype(f16).reshape(H, 2, 128, D)
    )
    bc_row = (bc * sig).astype(f16).reshape(1, D)
    ones_row = np.ones((1, 128), f16)
    ones_col = np.ones((128, 1), f16)
    ident = np.eye(128, dtype=f16)
    gamma_r = gamma.reshape(1, D).astype(f32)
    beta_r = beta.reshape(1, D).astype(f32)

    in_maps = []
    for c in range(NCORES):
        rs = slice(c * BC, (c + 1) * BC)
        ne_c = ne[rs].reshape(BC * K, D).astype(f16)
        neT_c = np.ascontiguousarray(ne_c.T).reshape(2, 128, BC * K)
        ceT_c = np.ascontiguousarray(ce[rs].astype(f16).T).reshape(2, 128, BC)
        nw_c = np.ascontiguousarray(
            nw[rs].reshape(NBT, NCH, 128).transpose(0, 2, 1).astype(f32)
        )
        va_c = np.ascontiguousarray(
            va[rs].reshape(NBT, NCH, 128).transpose(0, 2, 1).astype(f16)
        )
        in_maps.append({
            "neT": neT_c,
            "ner": ne_c,
            "ceT": ceT_c,
            "center": np.ascontiguousarray(ce[rs]),
            "nw_t": nw_c,
            "valid_t": va_c,
            "VU": vu_in,
            "U2": u2_in,
            "w2pair": w2pair,
            "b1cat": b1cat,
            "bm16": bm16,
            "Wcc": wcc,
            "bc_row": bc_row,
            "ones_row": ones_row,
            "ones_col": ones_col,
            "ident": ident,
            "gamma_r": gamma_r,
            "beta_r": beta_r,
        })

    trace = bool(os.environ.get("NE_KERNEL_TRACE"))
    if trace:
        _maybe_install_profile_hook()
    res = run_bass_kernel_spmd(nc, in_maps, list(range(NCORES)), trace=trace)
    LAST_EXEC_NS = res.exec_time_ns
    if trace:
        print("kernel exec_time_ns:", res.exec_time_ns, "mean:", res.mean_exec_time_ns)

    out = np.empty((B, D), f32)
    for c in range(NCORES):
        out[c * BC:(c + 1) * BC] = res.results[c]["out"]
    return out



# revision 11
# speedup vs baseline: 1.1172x; 1.1172x over previous
"""Trainium2 Bass kernel for nn_NeighborhoodAttentionModule.

Pure data-parallel over batch: B=16384 rows split as 2048 rows/core across 8
NeuronCores, 16 b-tiles of 128 rows per core. Per b-tile:

  s1T[b,(h,a)]   = ceT8.T @ U2         (PE DoubleRow fp8, 1 matmul)
  z[(h,a),(b,k)] = VU.T @ neT8 + s1T-broadcast   (PE: fp8 DoubleRow + fp16
                   rank-expand matmul accumulated into same PSUM)
  h = tanh(z)                          (ACT, PSUM->SBUF fp16)
  raw[(b,k),(c,h)] = h_chunk.T @ w2    (PE chunk-stationary, 16 tiny matmuls)
  rawn = raw + nwv                     (DVE; nwv = valid ? nw : -30, host-folded)
  em = exp(rawn)                       (ACT fp16; invalid -> exp(-30+raw) == 0)
  S[(b',h),(c,h)] via bm8 matmul; recS = 1/(S+2e-5)  (PE + DVE)
  recSmap = bm8T @ recS                (PE partition-broadcast)
  p = em * recSmap; expblk = p * blockmask           (DVE, fp16)
  aggT[dd,(b,h)] += ner8_chunk.T @ expblk_chunk      (PE, fp8 x fp16 mixed)
  fused = aggT.T @ Wcc + bc            (PE)
  out = LayerNorm(fused + center)      (DVE only: STT-accum stats + int
                                        rsqrt bithack + 2 Newton steps)

Zero ACT table swaps (Tanh+Exp share the exp_and_others table). All DRAM
tensors are laid out host-side as per-tile SBUF images (4KB/512B contiguous
partition lines) for dense DMA descriptors.
"""
import os
import numpy as np

B, K, D, H, A = 16384, 16, 256, 2, 64
NCORES = 8
BC = B // NCORES      # rows per core (2048)
NBT = BC // 128       # b-tiles per core (16)
NCH = 16              # chunks of 128 (b,k)-rows per b-tile
HA = H * A            # 128
EPS = 1e-5
NWV_NEG = -30.0       # folded invalid-neighbor bias
S_EPS = 2e-5          # S regularizer (valid rows always have S >= 1.5e-3)
RSQRT_MAGIC = 0x5F3759DF

LAST_EXEC_NS = None

_prog_cache = {}


def _maybe_install_profile_hook():
    """Optional NTFF profiling hook (for local testing only; fails soft)."""
    import sys, types, contextlib, ctypes

    if "antenv.axon_hooks" in sys.modules:
        return
    try:
        mod = types.ModuleType("antenv.axon_hooks")
        _state = {"hook": None}
        mod.set_axon_ntff_profile_hook = lambda h: _state.__setitem__("hook", h)
        mod.get_axon_ntff_profile_hook = lambda: _state["hook"]
        sys.modules["antenv.axon_hooks"] = mod
        import antenv

        antenv.axon_hooks = mod
        so_path = "/opt/axon/libaxon_pjrt.so"
        lib = ctypes.CDLL(so_path)
        if not hasattr(lib, "axon_start_nrt_profile"):
            return
        lib.axon_start_nrt_profile.argtypes = [
            ctypes.POINTER(ctypes.c_int64),
            ctypes.c_size_t,
        ]
        lib.axon_start_nrt_profile.restype = ctypes.c_int64
        lib.axon_stop_nrt_profile.argtypes = [ctypes.c_char_p]
        lib.axon_stop_nrt_profile.restype = ctypes.c_int64

        @contextlib.contextmanager
        def _hook(output_dir, device_ids):
            import jax

            jax.devices()
            if device_ids:
                ids = (ctypes.c_int64 * len(device_ids))(*device_ids)
                rc = lib.axon_start_nrt_profile(ids, len(device_ids))
            else:
                rc = lib.axon_start_nrt_profile(None, 0)
            if rc != 0:
                raise RuntimeError(f"axon_start_nrt_profile rc={rc}")
            try:
                yield
            finally:
                n = lib.axon_stop_nrt_profile(str(output_dir).encode())
                print(f"profile: {n} ntff file(s) -> {output_dir}")

        mod.set_axon_ntff_profile_hook(_hook)
    except Exception as e:  # noqa: BLE001
        print("profile hook unavailable:", e)


def _build_program(apply_gamma_beta: bool, apply_b1: bool):
    from concourse import bacc, tile, mybir

    F8 = mybir.dt.float8e4
    F16 = mybir.dt.float16
    F32 = mybir.dt.float32
    I32 = mybir.dt.int32
    AFT = mybir.ActivationFunctionType
    ALU = mybir.AluOpType
    PM = mybir.MatmulPerfMode

    nc = bacc.Bacc(None, target_bir_lowering=False)

    # ---- DRAM parameters (per-core shard; all per-tile SBUF images) ----
    dp = nc.declare_dram_parameter
    neT_d = dp("neT8", [NBT, 128, 2, 2048], F8, isOutput=False)
    ner_d = dp("ner8", [NBT, 128, NCH, D], F8, isOutput=False)
    ce_d = dp("ce16", [NBT, 128, D], F16, isOutput=False)
    nwv_d = dp("nwv16", [NBT, 128, NCH], F16, isOutput=False)
    ceT_d = dp("ceT8", [128, 2, BC], F8, isOutput=False)
    vu_d = dp("vu8", [128, 2, HA], F8, isOutput=False)
    u2_d = dp("u28", [128, 2, HA], F8, isOutput=False)
    w2_d = dp("w2p16", [128, H], F16, isOutput=False)
    b1_d = dp("b1row", [1, HA], F16, isOutput=False)
    wcc_d = dp("wcc16", [2, 2, 128, D], F16, isOutput=False)
    bcr_d = dp("bc_row", [1, D], F16, isOutput=False)
    onr_d = dp("ones_row", [1, 128], F16, isOutput=False)
    bm8_d = dp("bm8", [128, 8], F16, isOutput=False)
    bm8T_d = dp("bm8T", [8, 128], F16, isOutput=False)
    bm16_d = dp("bm16", [128, 8, H], F16, isOutput=False)
    idk_d = dp("ident32k", [32, 512], F16, isOutput=False)
    gam_d = dp("gamma_r", [1, D], F32, isOutput=False)
    bet_d = dp("beta_r", [1, D], F32, isOutput=False)
    out_d = dp("out", [BC, D], F32, isOutput=True)
    debug = bool(os.environ.get("NE_DEBUG_DUMP"))
    if debug:
        dbg_s1_d = dp("dbg_s1", [32, 4, HA], F16, isOutput=True)
        dbg_h_d = dp("dbg_h", [128, 2048], F16, isOutput=True)
        dbg_rawn_d = dp("dbg_rawn", [128, NCH, H], F32, isOutput=True)
        dbg_em_d = dp("dbg_em", [128, NCH, H], F16, isOutput=True)
        dbg_p_d = dp("dbg_p", [128, NCH, H], F16, isOutput=True)
        dbg_aggT_d = dp("dbg_aggT", [128, 2, 2 * 128], F16, isOutput=True)
        dbg_x_d = dp("dbg_x", [128, D], F32, isOutput=True)

    with tile.TileContext(nc) as tc:
        with (
            tc.tile_pool(name="const", bufs=1) as cpool,
            tc.tile_pool(name="loads", bufs=3) as lpool,
            tc.tile_pool(name="work", bufs=2) as wpool,
            tc.tile_pool(name="zps", bufs=2, space="PSUM") as zps_p,
            tc.tile_pool(name="srm_ps", bufs=1, space="PSUM") as srm_p,
            tc.tile_pool(name="aggps", bufs=1, space="PSUM") as aggps_p,
            tc.tile_pool(name="fups", bufs=1, space="PSUM") as fups_p,
        ):
            def cload(name, dram_ap, shape, dt):
                t = cpool.tile(shape, dt, tag=name, name=name)
                nc.sync.dma_start(t[:], dram_ap)
                return t

            ceT8 = cload("ceT8", ceT_d[:], [128, 2, BC], F8)
            vu8 = cload("vu8", vu_d[:], [128, 2, HA], F8)
            u28 = cload("u28", u2_d[:], [128, 2, HA], F8)
            w2p = cload("w2p", w2_d[:], [128, H], F16)
            b1r = cload("b1r", b1_d[:], [1, HA], F16) if apply_b1 else None
            wcc = [
                [cload(f"wcc{h}{dh}", wcc_d[h, dh], [128, D], F16) for dh in range(2)]
                for h in range(2)
            ]
            bcr = cload("bcr", bcr_d[:], [1, D], F16)
            ones_row = cload("onr", onr_d[:], [1, 128], F16)
            bm8 = cload("bm8", bm8_d[:], [128, 8], F16)
            bm8T = cload("bm8T", bm8T_d[:], [8, 128], F16)
            bm16 = cload("bm16", bm16_d[:], [128, 8, H], F16)
            idk = cload("idk", idk_d[:], [32, 512], F16)
            gam_t = (
                cload("gam", gam_d[:].to_broadcast((128, D)), [128, D], F32)
                if apply_gamma_beta else None
            )
            bet_t = (
                cload("bet", bet_d[:].to_broadcast((128, D)), [128, D], F32)
                if apply_gamma_beta else None
            )

            for t in range(NBT):
                # ---- loads ----
                neT = lpool.tile([128, 2, 2048], F8, tag="neT")
                nc.sync.dma_start(neT[:], neT_d[t])
                ner = lpool.tile([128, NCH, D], F8, tag="ner")
                nc.sync.dma_start(ner[:], ner_d[t])
                cen = lpool.tile([128, D], F16, tag="cen")
                nc.sync.dma_start(cen[:], ce_d[t])
                nwv = lpool.tile([128, NCH], F16, tag="nwv")
                nc.sync.dma_start(nwv[:], nwv_d[t])

                # ---- s1T[b%32, b//32, (h,a)] = ceT8_t.T @ U2 (+ b1 row) ----
                # partition-folded (4 groups of 32 b-rows at base partition 0)
                # because matmul operands may only start at partitions 0/32/64.
                # s1 lives in the fused-output bank; dead before fused starts.
                fub = fups_p.tile([128, 2 * D], F32, tag="fu")
                fu_ps = fub[:, 0:D]
                s1_ps = fub[0:32, :].rearrange("p (g f) -> p g f", g=4)
                for g in range(4):
                    nc.tensor.matmul(
                        s1_ps[:, g, :],
                        ceT8[:, :, t * 128 + g * 32:t * 128 + (g + 1) * 32],
                        u28[:],
                        start=True, stop=not apply_b1, perf_mode=PM.DoubleRow,
                        skip_group_check=True,
                    )
                    if apply_b1:
                        nc.tensor.matmul(
                            s1_ps[:, g, :], ones_row[:, 0:32], b1r[:],
                            start=False, stop=True, skip_group_check=True,
                        )
                s1_sb = wpool.tile([32, 4, HA], F16, tag="s1sb")
                nc.vector.tensor_copy(s1_sb[:], s1_ps)
                if debug and t == 0:
                    nc.sync.dma_start(dbg_s1_d[:], s1_sb[:])

                # ---- z = VU.T @ neT8 + s1-broadcast; tanh -> h ----
                # moving free dim caps at 512 -> 256-col pieces (DR moves 2x).
                h_sb = wpool.tile([128, 2048], F16, tag="h")
                for hf in range(2):
                    # normal-mode s1 expand FIRST (start), DoubleRow
                    # accumulates after: mixing perf modes the other way
                    # round corrupts the PSUM region.
                    z_ps = zps_p.tile([128, 1024], F32, tag="z")
                    for g in range(2):
                        gg = hf * 2 + g
                        nc.tensor.matmul(
                            z_ps[:, g * 512:(g + 1) * 512],
                            s1_sb[:, gg, :], idk[:],
                            start=True, stop=False, skip_group_check=True,
                        )
                    for m in range(4):
                        c0 = hf * 1024 + m * 256
                        nc.tensor.matmul(
                            z_ps[:, m * 256:(m + 1) * 256], vu8[:],
                            neT[:, :, c0:c0 + 256],
                            start=False, stop=True, perf_mode=PM.DoubleRow,
                            skip_group_check=True,
                        )
                    nc.scalar.activation(
                        h_sb[:, hf * 1024:(hf + 1) * 1024], z_ps[:], AFT.Tanh,
                    )

                if debug and t == 0:
                    nc.sync.dma_start(dbg_h_d[:], h_sb[:])
                # ---- raw scores (chunk-stationary) + nwv add ----
                # raw, S and recSmap share one PSUM bank; their lifetimes are
                # strictly sequential (raw -> rawn -> em -> S -> recS -> rmap)
                # and the byte overlap orders them.
                srm = srm_p.tile([128, NCH, H], F32, tag="srm")
                raw_ps = srm
                for c in range(NCH):
                    nc.tensor.matmul(
                        raw_ps[:, c, :],
                        h_sb[:, c * 128:(c + 1) * 128], w2p[:],
                        start=True, stop=True,
                    )
                rawn = wpool.tile([128, NCH, H], F32, tag="rawn")
                nc.vector.tensor_add(
                    rawn[:], raw_ps[:],
                    nwv[:, :, None].to_broadcast((128, NCH, H)),
                )

                if debug and t == 0:
                    nc.sync.dma_start(dbg_rawn_d[:], rawn[:])
                # ---- em = exp(rawn); S; recS; recSmap; p; expblk ----
                em = wpool.tile([128, NCH, H], F16, tag="em")
                nc.scalar.activation(
                    em[:].rearrange("p c h -> p (c h)"),
                    rawn[:].rearrange("p c h -> p (c h)"), AFT.Exp,
                )
                s_ps = srm[0:8].rearrange("p c h -> p (c h)")
                nc.tensor.matmul(
                    s_ps, bm8[:], em[:].rearrange("p c h -> p (c h)"),
                    start=True, stop=True,
                )
                s_eps = wpool.tile([8, NCH * H], F32, tag="seps")
                nc.vector.tensor_scalar_add(s_eps[:], s_ps, S_EPS)
                recS = wpool.tile([8, NCH * H], F16, tag="recS")
                with nc.allow_low_precision(reason="recS feeds fp16 matmul"):
                    nc.vector.reciprocal(recS[:], s_eps[:])
                rmap_ps = srm[:].rearrange("p c h -> p (c h)")
                nc.tensor.matmul(rmap_ps, bm8T[:], recS[:], start=True, stop=True)
                p_sb = wpool.tile([128, NCH, H], F16, tag="p")
                nc.vector.tensor_mul(
                    p_sb[:], em[:], srm[:],
                )
                if debug and t == 0:
                    nc.sync.dma_start(dbg_em_d[:], em[:])
                    nc.sync.dma_start(dbg_p_d[:], p_sb[:])
                expblk = wpool.tile([128, NCH, 8, H], F16, tag="expblk")
                nc.vector.tensor_mul(
                    expblk[:],
                    p_sb[:, :, None, :].to_broadcast((128, NCH, 8, H)),
                    bm16[:, None, :, :].to_broadcast((128, NCH, 8, H)),
                )

                # ---- aggT[dd, dh, (b,h)] += ner8_c.T @ expblk_c ----
                agg_ps = aggps_p.tile([128, 2, 2 * 128], F32, tag="aggT")
                for c in range(NCH):
                    for dh in range(2):
                        nc.tensor.matmul(
                            agg_ps[:, dh, 16 * c:16 * c + 16],
                            ner[:, c, dh * 128:(dh + 1) * 128],
                            expblk[:, c],
                            start=True, stop=True,
                        )
                aggT = wpool.tile([128, 2, 2 * 128], F16, tag="aggTsb")
                nc.vector.tensor_copy(aggT[:], agg_ps[:])

                if debug and t == 0:
                    nc.sync.dma_start(dbg_aggT_d[:], aggT[:])
                # ---- fused = combined @ Wc (+ bc row) ----
                mms = [(h, dh) for h in range(2) for dh in range(2)]
                for i, (h, dh) in enumerate(mms):
                    lhs = aggT[:, dh].rearrange("p (b h) -> p h b", h=2)[:, h, :]
                    nc.tensor.matmul(
                        fu_ps, lhs, wcc[h][dh][:],
                        start=(i == 0), stop=False, skip_group_check=True,
                    )
                nc.tensor.matmul(fu_ps, ones_row[:], bcr[:], start=False, stop=True,
                                 skip_group_check=True)

                # ---- residual + layernorm (all DVE) ----
                x_t = wpool.tile([128, D], F32, tag="x")
                msum = wpool.tile([128, 1], F32, tag="msum")
                nc.vector.scalar_tensor_tensor(
                    x_t[:], fu_ps, 1.0, cen[:],
                    op0=ALU.mult, op1=ALU.add, accum_out=msum[:],
                )
                if debug and t == 0:
                    nc.sync.dma_start(dbg_x_d[:], x_t[:])
                negmean = wpool.tile([128, 1], F32, tag="negmean")
                nc.vector.tensor_scalar_mul(negmean[:], msum[:], -1.0 / D)
                sq_t = wpool.tile([128, D], F32, tag="sq")
                sumsq = wpool.tile([128, 1], F32, tag="sumsq")
                nc.vector.scalar_tensor_tensor(
                    sq_t[:], x_t[:], 1.0, x_t[:],
                    op0=ALU.mult, op1=ALU.mult, accum_out=sumsq[:],
                )
                m2 = wpool.tile([128, 1], F32, tag="m2")
                nc.vector.tensor_mul(m2[:], negmean[:], negmean[:])
                q_t = wpool.tile([128, 1], F32, tag="q")
                nc.vector.tensor_scalar(
                    q_t[:], sumsq[:], 1.0 / D, EPS, op0=ALU.mult, op1=ALU.add,
                )
                nc.vector.tensor_sub(q_t[:], q_t[:], m2[:])
                # invstd = rsqrt(q): int bithack + 2 Newton steps
                yi = wpool.tile([128, 1], I32, tag="yi")
                nc.vector.tensor_scalar(
                    yi[:], q_t[:].bitcast(I32), 1, None,
                    op0=ALU.logical_shift_right,
                )
                nc.vector.tensor_scalar(
                    yi[:], yi[:], RSQRT_MAGIC, -1, op0=ALU.subtract, op1=ALU.mult,
                )
                y = yi[:].bitcast(F32)
                nr1 = wpool.tile([128, 1], F32, tag="nr1")
                nr2 = wpool.tile([128, 1], F32, tag="nr2")
                for _ in range(2):
                    nc.vector.tensor_mul(nr1[:], y, y)
                    nc.vector.scalar_tensor_tensor(
                        nr2[:], q_t[:], -0.5, nr1[:], op0=ALU.mult, op1=ALU.mult,
                    )
                    nc.vector.tensor_scalar(nr1[:], nr2[:], 1.5, None, op0=ALU.add)
                    nc.vector.tensor_mul(yi[:].bitcast(F32), y, nr1[:])
                xn = wpool.tile([128, D], F32, tag="xn")
                nc.vector.tensor_scalar(
                    xn[:], x_t[:], negmean[:], yi[:].bitcast(F32),
                    op0=ALU.add, op1=ALU.mult,
                )
                if apply_gamma_beta:
                    nc.vector.tensor_mul(xn[:], xn[:], gam_t[:])
                    nc.vector.tensor_add(xn[:], xn[:], bet_t[:])
                nc.sync.dma_start(out_d[t * 128:(t + 1) * 128, :], xn[:])

    nc.finalize()
    return nc


def _f8(x):
    import ml_dtypes
    return np.clip(x, -240.0, 240.0).astype(ml_dtypes.float8_e4m3)


def kernel(center_emb, neighbor_embs, neighbor_weights, neighbor_valid,
           W1, b1, w2, Wc, bc, alpha, gamma, beta):
    from concourse.bass_utils import run_bass_kernel_spmd

    global LAST_EXEC_NS

    f32 = np.float32
    f16 = np.float16
    ce = np.asarray(center_emb, f32)
    ne = np.asarray(neighbor_embs, f32)
    nw = np.asarray(neighbor_weights, f32)
    va = np.asarray(neighbor_valid)
    W1 = np.asarray(W1, f32)
    b1 = np.asarray(b1, f32)
    w2 = np.asarray(w2, f32)
    Wc = np.asarray(Wc, f32)
    bc = np.asarray(bc, f32)
    alpha = np.asarray(alpha, f32)
    gamma = np.asarray(gamma, f32)
    beta = np.asarray(beta, f32)

    apply_gamma_beta = not (np.all(gamma == 1.0) and np.all(beta == 0.0))
    apply_b1 = bool(np.any(b1 != 0.0))

    key = (apply_gamma_beta, apply_b1, bool(os.environ.get("NE_DEBUG_DUMP")))
    if key not in _prog_cache:
        _prog_cache[key] = _build_program(key[0], key[1])
    nc = _prog_cache[key]

    # ---- host-side const prep (weight folding + dtype casts + layouts) ----
    sig = 1.0 / (1.0 + np.exp(-float(alpha[0])))
    VU = np.concatenate([W1[h, D:2 * D] - W1[h, 2 * D:3 * D] for h in range(H)], axis=1)
    U2 = np.concatenate([W1[h, :D] + W1[h, 2 * D:3 * D] for h in range(H)], axis=1)
    # d = p + 128*i  ->  [p, i, cols]
    vu8 = np.ascontiguousarray(_f8(VU).reshape(2, 128, HA).transpose(1, 0, 2))
    u28 = np.ascontiguousarray(_f8(U2).reshape(2, 128, HA).transpose(1, 0, 2))
    w2p16 = np.zeros((128, H), f16)
    for h in range(H):
        w2p16[h * A:(h + 1) * A, h] = w2[h].astype(f16)
    b1row = b1.reshape(1, HA).astype(f16)
    wcc16 = np.ascontiguousarray((Wc * sig).astype(f16).reshape(H, 2, 128, D))
    bc_row = (bc * sig).astype(f16).reshape(1, D)
    ones_row = np.ones((1, 128), f16)
    pidx = np.arange(128)
    bm8 = (pidx[:, None] // 16 == np.arange(8)[None, :]).astype(f16)
    bm8T = np.ascontiguousarray(bm8.T)
    bm16 = np.zeros((128, 8, H), f16)
    for p in range(128):
        bm16[p, p // 16, :] = 1.0
    idk = np.zeros((32, 512), f16)
    for pl in range(32):
        idk[pl, pl * 16:(pl + 1) * 16] = 1.0
    gamma_r = gamma.reshape(1, D).astype(f32)
    beta_r = beta.reshape(1, D).astype(f32)

    nwv = np.where(va, nw, NWV_NEG).astype(f16)        # [B, K]

    in_maps = []
    for cidx in range(NCORES):
        rs = slice(cidx * BC, (cidx + 1) * BC)
        ne_c = _f8(ne[rs].reshape(BC * K, D))          # [BC*K, D] fp8
        # neT8 [t, p, i, col]: ne[row(t,col), p+128i]
        neT8 = np.ascontiguousarray(
            ne_c.reshape(NBT, 2048, 2, 128).transpose(0, 3, 2, 1)
        )
        # ner8 [t, p, c, d]: ne[t*2048 + c*128 + p, d]
        ner8 = np.ascontiguousarray(
            ne_c.reshape(NBT, NCH, 128, D).transpose(0, 2, 1, 3)
        )
        ce16 = np.ascontiguousarray(
            ce[rs].astype(f16).reshape(NBT, 128, D)
        )
        ceT8 = np.ascontiguousarray(
            _f8(ce[rs]).reshape(BC, 2, 128).transpose(2, 1, 0)
        )
        nwv16 = np.ascontiguousarray(
            nwv[rs].reshape(NBT, NCH, 128).transpose(0, 2, 1)
        )
        in_maps.append({
            "neT8": neT8,
            "ner8": ner8,
            "ce16": ce16,
            "nwv16": nwv16,
            "ceT8": ceT8,
            "vu8": vu8,
            "u28": u28,
            "w2p16": w2p16,
            "b1row": b1row,
            "wcc16": wcc16,
            "bc_row": bc_row,
            "ones_row": ones_row,
            "bm8": bm8,
            "bm8T": bm8T,
            "bm16": bm16,
            "ident32k": idk,
            "gamma_r": gamma_r,
            "beta_r": beta_r,
        })

    trace = bool(os.environ.get("NE_KERNEL_TRACE"))
    if trace:
        _maybe_install_profile_hook()
    res = run_bass_kernel_spmd(nc, in_maps, list(range(NCORES)), trace=trace)
    LAST_EXEC_NS = res.exec_time_ns
    if trace:
        print("kernel exec_time_ns:", res.exec_time_ns, "mean:", res.mean_exec_time_ns)

    out = np.empty((B, D), f32)
    for cidx in range(NCORES):
        out[cidx * BC:(cidx + 1) * BC] = res.results[cidx]["out"]
    return out


# revision 12
# speedup vs baseline: 1.5904x; 1.4236x over previous
"""Trainium2 Bass kernel for nn_NeighborhoodAttentionModule.

Pure data-parallel over batch: B=16384 rows split as 2048 rows/core across 8
NeuronCores, 16 b-tiles of 128 rows per core. Per b-tile:

  s1T[b,(h,a)]   = ceT8.T @ U2         (PE DoubleRow fp8, 1 matmul)
  z[(h,a),(b,k)] = VU.T @ neT8 + s1T-broadcast   (PE: fp8 DoubleRow + fp16
                   rank-expand matmul accumulated into same PSUM)
  h = tanh(z)                          (ACT, PSUM->SBUF fp16)
  raw[(b,k),(c,h)] = h_chunk.T @ w2    (PE chunk-stationary, 16 tiny matmuls)
  rawn = raw + nwv                     (DVE; nwv = valid ? nw : -30, host-folded)
  em = exp(rawn)                       (ACT fp16; invalid -> exp(-30+raw) == 0)
  S[(b',h),(c,h)] via bm8 matmul; recS = 1/(S+2e-5)  (PE + DVE)
  recSmap = bm8T @ recS                (PE partition-broadcast)
  p = em * recSmap; expblk = p * blockmask           (DVE, fp16)
  aggT[dd,(b,h)] += ner8_chunk.T @ expblk_chunk      (PE, fp8 x fp16 mixed)
  fused = aggT.T @ Wcc + bc            (PE)
  out = LayerNorm(fused + center)      (DVE only: STT-accum stats + int
                                        rsqrt bithack + 2 Newton steps)

Zero ACT table swaps (Tanh+Exp share the exp_and_others table). All DRAM
tensors are laid out host-side as per-tile SBUF images (4KB/512B contiguous
partition lines) for dense DMA descriptors.
"""
import os
import numpy as np

B, K, D, H, A = 16384, 16, 256, 2, 64
NCORES = 8
BC = B // NCORES      # rows per core (2048)
NBT = BC // 128       # b-tiles per core (16)
NCH = 16              # chunks of 128 (b,k)-rows per b-tile
HA = H * A            # 128
EPS = 1e-5
NWV_NEG = -30.0       # folded invalid-neighbor bias
S_EPS = 2e-5          # S regularizer (valid rows always have S >= 1.5e-3)
RSQRT_MAGIC = 0x5F3759DF

LAST_EXEC_NS = None

_prog_cache = {}


def _maybe_install_profile_hook():
    """Optional NTFF profiling hook (for local testing only; fails soft)."""
    import sys, types, contextlib, ctypes

    if "antenv.axon_hooks" in sys.modules:
        return
    try:
        mod = types.ModuleType("antenv.axon_hooks")
        _state = {"hook": None}
        mod.set_axon_ntff_profile_hook = lambda h: _state.__setitem__("hook", h)
        mod.get_axon_ntff_profile_hook = lambda: _state["hook"]
        sys.modules["antenv.axon_hooks"] = mod
        import antenv

        antenv.axon_hooks = mod
        so_path = "/opt/axon/libaxon_pjrt.so"
        lib = ctypes.CDLL(so_path)
        if not hasattr(lib, "axon_start_nrt_profile"):
            return
        lib.axon_start_nrt_profile.argtypes = [
            ctypes.POINTER(ctypes.c_int64),
            ctypes.c_size_t,
        ]
        lib.axon_start_nrt_profile.restype = ctypes.c_int64
        lib.axon_stop_nrt_profile.argtypes = [ctypes.c_char_p]
        lib.axon_stop_nrt_profile.restype = ctypes.c_int64

        @contextlib.contextmanager
        def _hook(output_dir, device_ids):
            import jax

            jax.devices()
            if device_ids:
                ids = (ctypes.c_int64 * len(device_ids))(*device_ids)
                rc = lib.axon_start_nrt_profile(ids, len(device_ids))
            else:
                rc = lib.axon_start_nrt_profile(None, 0)
            if rc != 0:
                raise RuntimeError(f"axon_start_nrt_profile rc={rc}")
            try:
                yield
            finally:
                n = lib.axon_stop_nrt_profile(str(output_dir).encode())
                print(f"profile: {n} ntff file(s) -> {output_dir}")

        mod.set_axon_ntff_profile_hook(_hook)
    except Exception as e:  # noqa: BLE001
        print("profile hook unavailable:", e)


def _build_program(apply_gamma_beta: bool, apply_b1: bool):
    from concourse import bacc, tile, mybir

    F8 = mybir.dt.float8e4
    F16 = mybir.dt.float16
    F32 = mybir.dt.float32
    I32 = mybir.dt.int32
    AFT = mybir.ActivationFunctionType
    ALU = mybir.AluOpType
    PM = mybir.MatmulPerfMode

    nc = bacc.Bacc(None, target_bir_lowering=False)

    # ---- DRAM parameters (per-core shard; all per-tile SBUF images) ----
    dp = nc.declare_dram_parameter
    neT_d = dp("neT8", [NBT, 128, 2, 2048], F8, isOutput=False)
    ner_d = dp("ner8", [NBT, 128, NCH, D], F8, isOutput=False)
    ce_d = dp("ce16", [NBT, 128, D], F16, isOutput=False)
    nwv_d = dp("nwv16", [NBT, 128, NCH], F16, isOutput=False)
    ceT_d = dp("ceT8", [128, 2, BC], F8, isOutput=False)
    vu_d = dp("vu8", [128, 2, HA], F8, isOutput=False)
    u2_d = dp("u28", [128, 2, HA], F8, isOutput=False)
    w2_d = dp("w2p16", [128, H], F16, isOutput=False)
    b1_d = dp("b1row", [1, HA], F16, isOutput=False)
    wcc_d = dp("wcc16", [2, 2, 128, D], F16, isOutput=False)
    bcr_d = dp("bc_row", [1, D], F16, isOutput=False)
    onr_d = dp("ones_row", [1, 128], F16, isOutput=False)
    bm8_d = dp("bm8", [128, 8], F16, isOutput=False)
    bm8T_d = dp("bm8T", [8, 128], F16, isOutput=False)
    bm16_d = dp("bm16", [128, 8, H], F16, isOutput=False)
    idk_d = dp("ident32k", [32, 512], F16, isOutput=False)
    gam_d = dp("gamma_r", [1, D], F32, isOutput=False)
    bet_d = dp("beta_r", [1, D], F32, isOutput=False)
    out_d = dp("out", [BC, D], F32, isOutput=True)
    debug = bool(os.environ.get("NE_DEBUG_DUMP"))
    if debug:
        dbg_s1_d = dp("dbg_s1", [32, 4, HA], F16, isOutput=True)
        dbg_h_d = dp("dbg_h", [128, 2048], F16, isOutput=True)
        dbg_rawn_d = dp("dbg_rawn", [128, NCH, H], F32, isOutput=True)
        dbg_em_d = dp("dbg_em", [128, NCH, H], F16, isOutput=True)
        dbg_p_d = dp("dbg_p", [128, NCH, H], F16, isOutput=True)
        dbg_aggT_d = dp("dbg_aggT", [128, 2, 2 * 128], F16, isOutput=True)
        dbg_x_d = dp("dbg_x", [128, D], F32, isOutput=True)

    with tile.TileContext(nc) as tc:
        with (
            tc.tile_pool(name="const", bufs=1) as cpool,
            tc.tile_pool(name="loads", bufs=3) as lpool,
            tc.tile_pool(name="work", bufs=2) as wpool,
            tc.tile_pool(name="zps", bufs=2, space="PSUM") as zps_p,
            tc.tile_pool(name="s1ps", bufs=2, space="PSUM") as s1ps_p,
            tc.tile_pool(name="srm_ps", bufs=2, space="PSUM") as srm_p,
            tc.tile_pool(name="aggps", bufs=1, space="PSUM") as aggps_p,
            tc.tile_pool(name="fups", bufs=1, space="PSUM") as fups_p,
        ):
            def cload(name, dram_ap, shape, dt):
                t = cpool.tile(shape, dt, tag=name, name=name)
                nc.sync.dma_start(t[:], dram_ap)
                return t

            ceT8 = cload("ceT8", ceT_d[:], [128, 2, BC], F8)
            vu8 = cload("vu8", vu_d[:], [128, 2, HA], F8)
            u28 = cload("u28", u2_d[:], [128, 2, HA], F8)
            w2p = cload("w2p", w2_d[:], [128, H], F16)
            b1r = cload("b1r", b1_d[:], [1, HA], F16) if apply_b1 else None
            wcc = [
                [cload(f"wcc{h}{dh}", wcc_d[h, dh], [128, D], F16) for dh in range(2)]
                for h in range(2)
            ]
            bcr = cload("bcr", bcr_d[:], [1, D], F16)
            ones_row = cload("onr", onr_d[:], [1, 128], F16)
            bm8 = cload("bm8", bm8_d[:], [128, 8], F16)
            bm8T = cload("bm8T", bm8T_d[:], [8, 128], F16)
            bm16 = cload("bm16", bm16_d[:], [128, 8, H], F16)
            idk = cload("idk", idk_d[:], [32, 512], F16)
            gam_t = (
                cload("gam", gam_d[:].to_broadcast((128, D)), [128, D], F32)
                if apply_gamma_beta else None
            )
            bet_t = (
                cload("bet", bet_d[:].to_broadcast((128, D)), [128, D], F32)
                if apply_gamma_beta else None
            )

            for t in range(NBT):
                # ---- loads ----
                neT = lpool.tile([128, 2, 2048], F8, tag="neT")
                nc.sync.dma_start(neT[:], neT_d[t])
                ner = lpool.tile([128, NCH, D], F8, tag="ner")
                nc.sync.dma_start(ner[:], ner_d[t])
                cen = lpool.tile([128, D], F16, tag="cen")
                nc.sync.dma_start(cen[:], ce_d[t])
                nwv = lpool.tile([128, NCH], F16, tag="nwv")
                nc.sync.dma_start(nwv[:], nwv_d[t])

                # ---- s1T[b%32, b//32, (h,a)] = ceT8_t.T @ U2 (+ b1 row) ----
                # partition-folded (4 groups of 32 b-rows at base partition 0)
                # because matmul operands may only start at partitions 0/32/64.
                s1t = s1ps_p.tile([32, 4, HA], F32, tag="s1ps")
                s1_ps = s1t[:]
                for g in range(4):
                    nc.tensor.matmul(
                        s1_ps[:, g, :],
                        ceT8[:, :, t * 128 + g * 32:t * 128 + (g + 1) * 32],
                        u28[:],
                        start=True, stop=not apply_b1, perf_mode=PM.DoubleRow,
                        skip_group_check=True,
                    )
                    if apply_b1:
                        nc.tensor.matmul(
                            s1_ps[:, g, :], ones_row[:, 0:32], b1r[:],
                            start=False, stop=True, skip_group_check=True,
                        )
                s1_sb = wpool.tile([32, 4, HA], F16, tag="s1sb")
                nc.vector.tensor_copy(s1_sb[:], s1_ps)
                if debug and t == 0:
                    nc.sync.dma_start(dbg_s1_d[:], s1_sb[:])

                # ---- z = VU.T @ neT8 + s1-broadcast; tanh -> h ----
                # 512-col quarters, one PSUM bank each, fully double-buffered.
                # normal-mode s1 expand FIRST (start), DoubleRow accumulates
                # after: mixing perf modes the other way round corrupts PSUM.
                h_sb = wpool.tile([128, 2048], F16, tag="h")
                for q in range(4):
                    z_ps = zps_p.tile([128, 512], F32, tag="z")
                    nc.tensor.matmul(
                        z_ps[:], s1_sb[:, q, :], idk[:],
                        start=True, stop=False, skip_group_check=True,
                    )
                    for m in range(2):
                        c0 = q * 512 + m * 256
                        nc.tensor.matmul(
                            z_ps[:, m * 256:(m + 1) * 256], vu8[:],
                            neT[:, :, c0:c0 + 256],
                            start=False, stop=True, perf_mode=PM.DoubleRow,
                            skip_group_check=True,
                        )
                    nc.scalar.activation(
                        h_sb[:, q * 512:(q + 1) * 512], z_ps[:], AFT.Tanh,
                    )

                if debug and t == 0:
                    nc.sync.dma_start(dbg_h_d[:], h_sb[:])
                # ---- raw scores (chunk-stationary) + nwv add ----
                # raw, S and recSmap share one PSUM bank; their lifetimes are
                # strictly sequential (raw -> rawn -> em -> S -> recS -> rmap)
                # and the byte overlap orders them.
                srm = srm_p.tile([128, NCH, H], F32, tag="srm")
                raw_ps = srm
                for c in range(NCH):
                    nc.tensor.matmul(
                        raw_ps[:, c, :],
                        h_sb[:, c * 128:(c + 1) * 128], w2p[:],
                        start=True, stop=True,
                    )
                rawn = wpool.tile([128, NCH, H], F32, tag="rawn")
                nc.vector.tensor_add(
                    rawn[:], raw_ps[:],
                    nwv[:, :, None].to_broadcast((128, NCH, H)),
                )

                if debug and t == 0:
                    nc.sync.dma_start(dbg_rawn_d[:], rawn[:])
                # ---- em = exp(rawn); S; recS; recSmap; p; expblk ----
                em = wpool.tile([128, NCH, H], F16, tag="em")
                nc.scalar.activation(
                    em[:].rearrange("p c h -> p (c h)"),
                    rawn[:].rearrange("p c h -> p (c h)"), AFT.Exp,
                )
                s_ps = srm[0:8].rearrange("p c h -> p (c h)")
                nc.tensor.matmul(
                    s_ps, bm8[:], em[:].rearrange("p c h -> p (c h)"),
                    start=True, stop=True,
                )
                s_eps = wpool.tile([8, NCH * H], F32, tag="seps")
                nc.vector.tensor_scalar_add(s_eps[:], s_ps, S_EPS)
                recS = wpool.tile([8, NCH * H], F16, tag="recS")
                with nc.allow_low_precision(reason="recS feeds fp16 matmul"):
                    nc.vector.reciprocal(recS[:], s_eps[:])
                rmap_ps = srm[:].rearrange("p c h -> p (c h)")
                nc.tensor.matmul(rmap_ps, bm8T[:], recS[:], start=True, stop=True)
                p_sb = wpool.tile([128, NCH, H], F16, tag="p")
                nc.vector.tensor_mul(
                    p_sb[:], em[:], srm[:],
                )
                if debug and t == 0:
                    nc.sync.dma_start(dbg_em_d[:], em[:])
                    nc.sync.dma_start(dbg_p_d[:], p_sb[:])
                expblk = wpool.tile([128, NCH, 8, H], F16, tag="expblk")
                nc.vector.tensor_mul(
                    expblk[:],
                    p_sb[:, :, None, :].to_broadcast((128, NCH, 8, H)),
                    bm16[:, None, :, :].to_broadcast((128, NCH, 8, H)),
                )

                # ---- aggT[dd, dh, (b,h)] += ner8_c.T @ expblk_c ----
                agg_ps = aggps_p.tile([128, 2, 2 * 128], F32, tag="aggT")
                for c in range(NCH):
                    for dh in range(2):
                        nc.tensor.matmul(
                            agg_ps[:, dh, 16 * c:16 * c + 16],
                            ner[:, c, dh * 128:(dh + 1) * 128],
                            expblk[:, c],
                            start=True, stop=True,
                        )
                aggT = wpool.tile([128, 2, 2 * 128], F16, tag="aggTsb")
                nc.scalar.copy(aggT[:], agg_ps[:])

                if debug and t == 0:
                    nc.sync.dma_start(dbg_aggT_d[:], aggT[:])
                # ---- fused = combined @ Wc (+ bc row) ----
                fu_ps = fups_p.tile([128, D], F32, tag="fu")
                mms = [(h, dh) for h in range(2) for dh in range(2)]
                for i, (h, dh) in enumerate(mms):
                    lhs = aggT[:, dh].rearrange("p (b h) -> p h b", h=2)[:, h, :]
                    nc.tensor.matmul(
                        fu_ps[:], lhs, wcc[h][dh][:],
                        start=(i == 0), stop=False,
                    )
                nc.tensor.matmul(fu_ps[:], ones_row[:], bcr[:], start=False, stop=True)

                # ---- residual + layernorm (all DVE) ----
                x_t = wpool.tile([128, D], F32, tag="x")
                msum = wpool.tile([128, 1], F32, tag="msum")
                nc.vector.scalar_tensor_tensor(
                    x_t[:], fu_ps[:], 1.0, cen[:],
                    op0=ALU.mult, op1=ALU.add, accum_out=msum[:],
                )
                if debug and t == 0:
                    nc.sync.dma_start(dbg_x_d[:], x_t[:])
                negmean = wpool.tile([128, 1], F32, tag="negmean")
                nc.vector.tensor_scalar_mul(negmean[:], msum[:], -1.0 / D)
                sq_t = wpool.tile([128, D], F32, tag="sq")
                sumsq = wpool.tile([128, 1], F32, tag="sumsq")
                nc.vector.scalar_tensor_tensor(
                    sq_t[:], x_t[:], 1.0, x_t[:],
                    op0=ALU.mult, op1=ALU.mult, accum_out=sumsq[:],
                )
                m2 = wpool.tile([128, 1], F32, tag="m2")
                nc.vector.tensor_mul(m2[:], negmean[:], negmean[:])
                q_t = wpool.tile([128, 1], F32, tag="q")
                nc.vector.tensor_scalar(
                    q_t[:], sumsq[:], 1.0 / D, EPS, op0=ALU.mult, op1=ALU.add,
                )
                nc.vector.tensor_sub(q_t[:], q_t[:], m2[:])
                # invstd = rsqrt(q): int bithack + 2 Newton steps
                yi = wpool.tile([128, 1], I32, tag="yi")
                nc.vector.tensor_scalar(
                    yi[:], q_t[:].bitcast(I32), 1, None,
                    op0=ALU.logical_shift_right,
                )
                nc.vector.tensor_scalar(
                    yi[:], yi[:], RSQRT_MAGIC, -1, op0=ALU.subtract, op1=ALU.mult,
                )
                y = yi[:].bitcast(F32)
                nr1 = wpool.tile([128, 1], F32, tag="nr1")
                nr2 = wpool.tile([128, 1], F32, tag="nr2")
                for _ in range(2):
                    nc.vector.tensor_mul(nr1[:], y, y)
                    nc.vector.scalar_tensor_tensor(
                        nr2[:], q_t[:], -0.5, nr1[:], op0=ALU.mult, op1=ALU.mult,
                    )
                    nc.vector.tensor_scalar(nr1[:], nr2[:], 1.5, None, op0=ALU.add)
                    nc.vector.tensor_mul(yi[:].bitcast(F32), y, nr1[:])
                xn = wpool.tile([128, D], F32, tag="xn")
                nc.vector.tensor_scalar(
                    xn[:], x_t[:], negmean[:], yi[:].bitcast(F32),
                    op0=ALU.add, op1=ALU.mult,
                )
                if apply_gamma_beta:
                    nc.vector.tensor_mul(xn[:], xn[:], gam_t[:])
                    nc.vector.tensor_add(xn[:], xn[:], bet_t[:])
                nc.sync.dma_start(out_d[t * 128:(t + 1) * 128, :], xn[:])

    nc.finalize()
    return nc


def _f8(x):
    import ml_dtypes
    return np.clip(x, -240.0, 240.0).astype(ml_dtypes.float8_e4m3)


def kernel(center_emb, neighbor_embs, neighbor_weights, neighbor_valid,
           W1, b1, w2, Wc, bc, alpha, gamma, beta):
    from concourse.bass_utils import run_bass_kernel_spmd

    global LAST_EXEC_NS

    f32 = np.float32
    f16 = np.float16
    ce = np.asarray(center_emb, f32)
    ne = np.asarray(neighbor_embs, f32)
    nw = np.asarray(neighbor_weights, f32)
    va = np.asarray(neighbor_valid)
    W1 = np.asarray(W1, f32)
    b1 = np.asarray(b1, f32)
    w2 = np.asarray(w2, f32)
    Wc = np.asarray(Wc, f32)
    bc = np.asarray(bc, f32)
    alpha = np.asarray(alpha, f32)
    gamma = np.asarray(gamma, f32)
    beta = np.asarray(beta, f32)

    apply_gamma_beta = not (np.all(gamma == 1.0) and np.all(beta == 0.0))
    apply_b1 = bool(np.any(b1 != 0.0))

    key = (apply_gamma_beta, apply_b1, bool(os.environ.get("NE_DEBUG_DUMP")))
    if key not in _prog_cache:
        _prog_cache[key] = _build_program(key[0], key[1])
    nc = _prog_cache[key]

    # ---- host-side const prep (weight folding + dtype casts + layouts) ----
    sig = 1.0 / (1.0 + np.exp(-float(alpha[0])))
    VU = np.concatenate([W1[h, D:2 * D] - W1[h, 2 * D:3 * D] for h in range(H)], axis=1)
    U2 = np.concatenate([W1[h, :D] + W1[h, 2 * D:3 * D] for h in range(H)], axis=1)
    # d = p + 128*i  ->  [p, i, cols]
    vu8 = np.ascontiguousarray(_f8(VU).reshape(2, 128, HA).transpose(1, 0, 2))
    u28 = np.ascontiguousarray(_f8(U2).reshape(2, 128, HA).transpose(1, 0, 2))
    w2p16 = np.zeros((128, H), f16)
    for h in range(H):
        w2p16[h * A:(h + 1) * A, h] = w2[h].astype(f16)
    b1row = b1.reshape(1, HA).astype(f16)
    wcc16 = np.ascontiguousarray((Wc * sig).astype(f16).reshape(H, 2, 128, D))
    bc_row = (bc * sig).astype(f16).reshape(1, D)
    ones_row = np.ones((1, 128), f16)
    pidx = np.arange(128)
    bm8 = (pidx[:, None] // 16 == np.arange(8)[None, :]).astype(f16)
    bm8T = np.ascontiguousarray(bm8.T)
    bm16 = np.zeros((128, 8, H), f16)
    for p in range(128):
        bm16[p, p // 16, :] = 1.0
    idk = np.zeros((32, 512), f16)
    for pl in range(32):
        idk[pl, pl * 16:(pl + 1) * 16] = 1.0
    gamma_r = gamma.reshape(1, D).astype(f32)
    beta_r = beta.reshape(1, D).astype(f32)

    nwv = np.where(va, nw, NWV_NEG).astype(f16)        # [B, K]

    in_maps = []
    for cidx in range(NCORES):
        rs = slice(cidx * BC, (cidx + 1) * BC)
        ne_c = _f8(ne[rs].reshape(BC * K, D))          # [BC*K, D] fp8
        # neT8 [t, p, i, col]: ne[row(t,col), p+128i]
        neT8 = np.ascontiguousarray(
            ne_c.reshape(NBT, 2048, 2, 128).transpose(0, 3, 2, 1)
        )
        # ner8 [t, p, c, d]: ne[t*2048 + c*128 + p, d]
        ner8 = np.ascontiguousarray(
            ne_c.reshape(NBT, NCH, 128, D).transpose(0, 2, 1, 3)
        )
        ce16 = np.ascontiguousarray(
            ce[rs].astype(f16).reshape(NBT, 128, D)
        )
        ceT8 = np.ascontiguousarray(
            _f8(ce[rs]).reshape(BC, 2, 128).transpose(2, 1, 0)
        )
        nwv16 = np.ascontiguousarray(
            nwv[rs].reshape(NBT, NCH, 128).transpose(0, 2, 1)
        )
        in_maps.append({
            "neT8": neT8,
            "ner8": ner8,
            "ce16": ce16,
            "nwv16": nwv16,
            "ceT8": ceT8,
            "vu8": vu8,
            "u28": u28,
            "w2p16": w2p16,
            "b1row": b1row,
            "wcc16": wcc16,
            "bc_row": bc_row,
            "ones_row": ones_row,
            "bm8": bm8,
            "bm8T": bm8T,
            "bm16": bm16,
            "ident32k": idk,
            "gamma_r": gamma_r,
            "beta_r": beta_r,
        })

    trace = bool(os.environ.get("NE_KERNEL_TRACE"))
    if trace:
        _maybe_install_profile_hook()
    res = run_bass_kernel_spmd(nc, in_maps, list(range(NCORES)), trace=trace)
    LAST_EXEC_NS = res.exec_time_ns
    if trace:
        print("kernel exec_time_ns:", res.exec_time_ns, "mean:", res.mean_exec_time_ns)

    out = np.empty((B, D), f32)
    for cidx in range(NCORES):
        out[cidx * BC:(cidx + 1) * BC] = res.results[cidx]["out"]
    return out
